# revision 71
# baseline (speedup 1.0000x reference)
"""Causal self-attention kernel for 8 Trainium2 NeuronCores.

Problem: B=2, T=2048, C=1024, H=16 heads (HD=64).
  qkv = x @ w_attn + b_attn ; causal softmax attention ; y @ w_proj + b_proj

Sharding: tensor-parallel over heads. Core c owns heads {2c, 2c+1} for both
batches. Each core computes Q/K/V for its heads (from full x), runs causal
attention, and produces a partial projection output
outT_c = (y_local @ w_proj[rows_c])^T in bf16. Host sums the 8 partials,
adds the bias, and transposes back.

Design notes (activations/weights bf16, PSUM accumulation f32; rel err
~4e-3 vs the 2e-2 gate):
  - x is passed host-transposed as xT [C, B*T] (bf16) so it streams as the
    moving operand of qkvT = w_sel^T @ xT. Only the Q bias is applied on
    device: the K bias is a softmax no-op (per-query constant), and the V
    bias is folded into b_proj on the host (softmax weights sum to 1).
  - V is computed directly in natural [token, head-dim] layout using x as
    the matmul stationary operand (no PE transposes), and lands in a
    per-batch v_aug [128, kb, head, 65] tile whose 65th column is ones so
    the O' matmul also produces the softmax denominators for free.
  - Attention uses the S^T layout: S^T[k,q] PSUM tiles [128, q-span<=512];
    exp runs on ACT straight into a per-kb bf16 pT [128, 2 heads, span]
    tile; the causal diag mask is one multiplicative DVE op per kb; no
    max-subtraction (scores are O(1), exp stays finite).
  - normalize: DVE reciprocal of the denominator row, GPSIMD
    partition_broadcast across the 64 y-rows (no PE in the chain, so
    queued normalize units cannot deadlock the in-order PE stream), DVE
    multiply. Head 1's multiply writes yT[64:128] via a partition-shifted
    output AP. The final chunk uses an idle-PE matmul broadcast instead.
  - Scheduling: attention(0) starts after only tch0/1 of qkv(0); the rest
    of qkv(0), all of qkv(1), and (held-back) projection units drain as
    fillers inside the attention loops under tuned rates, keeping PE fed
    where ACT (exp) is locally the bottleneck. outT stores pair two
    128-row blocks per DMA to halve SP dispatch serialization.
"""

import numpy as np

B, T, C, H = 2, 2048, 1024, 16
HD = C // H          # 64
NCORES = 8
HPC = H // NCORES    # 2 heads per core
BT = B * T           # 4096
NCB = C // 128       # 8 contraction blocks
NKB = T // 128       # 16 key blocks per batch
NJC = T // 512       # 4 query chunks of 512 per batch

_CACHE = {}


def _build_program():
    import collections

    import concourse.bacc as bacc
    import concourse.mybir as mybir
    import concourse.tile as tile
    from concourse.masks import make_upper_triangular

    f32 = mybir.dt.float32
    f32r = mybir.dt.float32r
    bf16 = mybir.dt.bfloat16
    Exp = mybir.ActivationFunctionType.Exp

    nc = bacc.Bacc("TRN2", target_bir_lowering=False, debug=False,
                   num_devices=NCORES)

    xT_d = nc.dram_tensor("xT", [C, BT], bf16, kind="ExternalInput")
    wqkv_d = nc.dram_tensor("wqkv", [C, 3 * 128], bf16, kind="ExternalInput")
    bqkv_d = nc.dram_tensor("bqkv", [128, 3], f32, kind="ExternalInput")
    wp_d = nc.dram_tensor("wp", [128, C], bf16, kind="ExternalInput")
    outT_d = nc.dram_tensor("outT", [C, BT], bf16, kind="ExternalOutput")

    with tile.TileContext(nc) as tc:
        with tc.tile_pool(name="const", bufs=1) as cst, \
             tc.tile_pool(name="big", bufs=1) as big, \
             tc.tile_pool(name="work", bufs=2) as work, \
             tc.tile_pool(name="pwork", bufs=3) as pwork, \
             tc.tile_pool(name="ps", bufs=1, space="PSUM") as ps:

            # ---- critical-path loads, in consumption order ----
            w_sb = cst.tile([128, NCB, 3 * 128], bf16, tag="w")
            _wr = wqkv_d.ap().rearrange("(cb p) n -> p cb n", p=128)
            nc.sync.dma_start(w_sb[:, 0:1, :], _wr[:, 0:1, :])

            xT_r = xT_d.ap().rearrange("(cb p) t -> p cb t", p=128)

            # first x chunk (tch0) sub0, then the rest of w (needed by the
            # 2nd matmul of the first accumulation), then tch0 sub1
            x0_sb = work.tile([128, NCB, 512], bf16, tag="x", bufs=3,
                              name="x0")
            # first x chunks ride the (startup-idle) ACT HWDGE queue so
            # their dispatch chain overlaps the weight loads on SP
            nc.scalar.dma_start(x0_sb[:, 0:1, 0:256], xT_r[:, 0:1, 0:256])
            nc.sync.dma_start(w_sb[:, 1:4, :], _wr[:, 1:4, :])
            nc.sync.dma_start(w_sb[:, 4:NCB, :], _wr[:, 4:NCB, :])
            nc.sync.dma_start(x0_sb[:, 1:4, 0:256], xT_r[:, 1:4, 0:256])
            nc.sync.dma_start(x0_sb[:, 4:NCB, 0:256], xT_r[:, 4:NCB, 0:256])
            bq_sb = cst.tile([128, 3], f32, tag="bq")
            nc.sync.dma_start(bq_sb[:], bqkv_d.ap())
            nc.sync.dma_start(x0_sb[:, :, 256:512], xT_r[:, :, 256:512])

            # ---- remaining constants (wp load deferred to post-qkv) ----
            wp_sb = cst.tile([128, NCB, 128], bf16, tag="wp")
            maskm_f = cst.tile([128, 128], f32, tag="maskmf")
            make_upper_triangular(nc, maskm_f[:], val=1.0, diag=True)
            # two adjacent copies so the h-merged [128, 2, 128] diag
            # multiply uses one contiguous operand
            maskm = cst.tile([128, 2, 128], bf16, tag="maskm")
            nc.vector.tensor_copy(maskm[:, 0, :], maskm_f[:])
            nc.vector.tensor_copy(maskm[:, 1, :], maskm_f[:])
            onecol_f = cst.tile([128, 1], f32, tag="onecol")
            nc.vector.memset(onecol_f[:], 1.0)
            ones64 = cst.tile([1, 64], f32, tag="ones64")
            nc.vector.memset(ones64[:], 1.0)
            # prewarm the ACT exp table set while ACT is otherwise idle,
            # so the ~2.7us table load is off the attention critical path
            warm = cst.tile([1, 2], f32, tag="warm")
            nc.scalar.activation(warm[:, 0:1], onecol_f[0:1, 0:1], Exp)

            # ---- persistent activations ----
            qkvT = [big.tile([128, BT], bf16, tag=f"qkvT{t}", name=f"qkvT{t}")
                    for t in range(2)]
            yT = big.tile([128, BT], bf16, tag="yT", name="yT")

            # K bias is dropped entirely (softmax is invariant to the
            # per-query constant q . bk), and the V bias is folded into
            # b_proj on the host (y = y_attn + bv exactly, softmax weights
            # sum to 1), so only the Q bias is applied on-device.
            def qkv_units(b, tchs=None):
                for tch in (tchs if tchs is not None
                            else range(4 * b, 4 * b + 4)):
                    tc0 = tch * 512
                    if tch == 0:
                        x_sb = x0_sb       # DMA already emitted above
                    else:
                        x_sb = work.tile([128, NCB, 512], bf16, tag="x",
                                         bufs=3, name=f"x{tch}")
                        for s in range(2):
                            nc.sync.dma_start(
                                x_sb[:, :, s * 256:(s + 1) * 256],
                                xT_r[:, :, tc0 + s * 256:tc0 + (s + 1) * 256])
                    split = 2 if tch == 0 else 1
                    sub = 512 // split
                    yield
                    for cht in range(2):
                        pq = ps.tile([128, 512], f32, tag="sps", bufs=4,
                                     name=f"pq{tch}{cht}")
                        for s in range(split):
                            for cb in range(NCB):
                                nc.tensor.matmul(
                                    pq[:, s * sub:(s + 1) * sub],
                                    w_sb[:, cb, cht * 128:(cht + 1) * 128],
                                    x_sb[:, cb, s * sub:(s + 1) * sub],
                                    start=(cb == 0), stop=(cb == NCB - 1))
                        if cht == 0:
                            nc.vector.tensor_scalar_add(
                                qkvT[0][:, tc0:tc0 + 512], pq[:],
                                bq_sb[:, 0:1])
                        else:
                            nc.vector.tensor_copy(
                                qkvT[1][:, tc0:tc0 + 512], pq[:])
                        yield
                    # V in natural [token, head-dim] layout: x as the
                    # stationary operand, wv as moving -> no PE transposes
                    pv = ps.tile([128, 4, 2, 64], f32, tag="sps", bufs=4,
                                 name=f"pv{tch}")
                    for blk in range(4):
                        for cb in range(NCB):
                            nc.tensor.matmul(
                                pv[:, blk, :, :],
                                x_sb[:, cb, blk * 128:(blk + 1) * 128],
                                w_sb[:, cb, 2 * 128:3 * 128],
                                start=(cb == 0), stop=(cb == NCB - 1))
                        yield
                    kb0 = (tch % 4) * 4
                    for blk in range(4):
                        nc.vector.tensor_copy(
                            v_aug[b][:, kb0 + blk, :, 0:64],
                            pv[:, blk, :, :])
                    yield

            def qkv_batch(b, tchs=None):
                for _ in qkv_units(b, tchs):
                    pass

            def proj_tile_units(b, tch, tail=False):
                # two 128-row output blocks share one osb tile and one DMA
                # (halves the SP dispatch serialization, tail especially)
                tc0 = tch * 512
                o_r = outT_d.ap().rearrange("(ob p) t -> p ob t", p=128)
                for ot in range(NCB):
                    if ot % 2 == 0:
                        osb = work.tile([128, 2, 512], bf16, tag="osb",
                                        bufs=6, name=f"osb{ot}{tch}")
                    pp = ps.tile([128, 512], f32, tag="sps", bufs=4,
                                 name=f"pp{ot}{tch}")
                    nc.tensor.matmul(pp[:], wp_sb[:, ot, :],
                                     yT[:, tc0:tc0 + 512],
                                     start=True, stop=True)
                    if tail and ot % 2 == 0:
                        # ACT is idle at the very end; splitting the copies
                        # across ACT/DVE halves the tail's serial chain
                        nc.scalar.copy(osb[:, ot % 2, :], pp[:])
                    else:
                        nc.vector.tensor_copy(osb[:, ot % 2, :], pp[:])
                    if ot % 2 == 1:
                        nc.sync.dma_start(
                            o_r[:, ot - 1:ot + 1, tc0:tc0 + 512],
                            osb[:])
                    yield

            def norm_units(b, o_ps, ocol, jc, tail=False):
                # normalize both heads, then hand the projection tiles to
                # projq (appending only after both normalizes are fully
                # emitted keeps the engine streams deadlock-free)
                for h in (0, 1):
                    normalize_jc(b, h, o_ps[h], ocol, jc, tail=tail)
                    yield
                projq.append(proj_tile_units(b, 4 * b + jc, tail=tail))

            def normalize_jc(b, h, o_ps, ocol, jc, c0=0, cw=512,
                             tail=False):
                # y^T cols [512jc, 512jc+512) (batch-rel) = O^T * (1/d).
                # Both heads accumulate at PSUM rows 0..64 (y + denom);
                # head 1's final multiply writes yT[64:128] via a
                # partition-shifted output AP (no partition-shift DMA).
                # The 1/d broadcast across the 64 y-rows runs on the idle
                # GPSIMD engine, so normalize uses no PE instructions at
                # all (keeps the queued norm+proj units deadlock-free).
                base = 2048 * b + 512 * jc + c0
                ylo = 0 if h == 0 else 64
                d_sb = work.tile([1, 512], f32, tag="dsb", bufs=4,
                                 name=f"d{b}{h}{jc}{c0}")
                with nc.allow_low_precision(
                        reason="softmax denominators (~1e-4)"):
                    nc.vector.reciprocal(d_sb[0:1, 0:cw],
                                         o_ps[64:65, ocol:ocol + cw])
                rec_sb = work.tile([64, 512], f32, tag="recsb", bufs=4,
                                   name=f"rec{b}{h}{jc}{c0}")
                if tail:
                    # PE and ACT are idle at the very end: broadcast 1/d via
                    # a PE matmul + ACT copy instead of the serial Pool path
                    recD = ps.tile([128, 512], f32, tag="sps", bufs=4,
                                   name=f"recD{b}{h}{jc}")
                    nc.tensor.matmul(recD[0:64, 0:cw], ones64[:],
                                     d_sb[0:1, 0:cw], start=True, stop=True)
                    nc.scalar.copy(rec_sb[0:64, 0:cw], recD[0:64, 0:cw])
                else:
                    nc.gpsimd.partition_broadcast(rec_sb[0:64, 0:cw],
                                                  d_sb[0:1, 0:cw])
                nc.vector.tensor_mul(
                    yT[ylo:ylo + 64, base:base + cw],
                    o_ps[0:64, ocol:ocol + cw], rec_sb[0:64, 0:cw])

            def drain_one(q):
                while q:
                    try:
                        next(q[0])
                        return True
                    except StopIteration:
                        q.popleft()
                return False

            def drain_q(q, n):
                for _ in range(n):
                    if not drain_one(q):
                        break

            def drain_fillers(n, proj_ok=True):
                # normalize units first (they release PSUM accumulators and
                # unblock downstream proj); then fillers; proj units only
                # when allowed -- holding proj back during attn(0) reserves
                # PE work for the filler-starved attn(1) windows
                for i in range(n):
                    if drain_one(normq):
                        continue
                    if drain_one(fillerq0):
                        continue
                    if drain_one(fillerq):
                        continue
                    if not (proj_ok and drain_one(projq)):
                        break

            def attn_batch(b, fill_rates=(1, 3), proj_ok=True,
                           halves=(0, 1), last_split=1):
                # Both heads processed together: head0 in PE rows 0-63,
                # head1 in rows 64-127 -> S matmul pairs run concurrently.
                # Query range split in two halves so both heads' O'
                # accumulators fit in PSUM (2 banks each).
                base = 2048 * b
                pend = []
                for hi, half in enumerate(halves):
                    if hi == 1:
                        # the second query half reads Q columns produced by
                        # the deferred qkv units -- emit them all first
                        drain_q(fillerq0, 10 ** 6)
                    q0 = 1024 * half
                    o_ps = [ps.tile([128, 1024], f32, tag="ops", bufs=2,
                                    name=f"o{b}{half}{h}") for h in (0, 1)]

                    def emit_o(kb, pT, o_ps=o_ps, q0=q0, half=half, hi=hi):
                        # default-arg binding: a pend entry carried across
                        # the half boundary must keep ITS half's state
                        span_lo = max(q0, 128 * kb)
                        for h in (0, 1):
                            for jc in range(max(2 * half, kb // 4),
                                            2 * half + 2):
                                cs = max(512 * jc, 128 * kb)
                                width = 512 * (jc + 1) - cs
                                nc.tensor.matmul(
                                    o_ps[h][0:65, cs - q0:cs - q0 + width],
                                    v_aug[b][:, kb, h, :],
                                    pT[:, h, cs - span_lo:
                                       cs - span_lo + width],
                                    start=(kb == 0), stop=(kb == 4 * jc + 3))
                        if kb % 4 == 3:
                            jc_done = kb // 4
                            if jc_done >= 2 * half:
                                is_last = (b == 1 and hi == 1
                                           and kb == 8 * half + 7)
                                normq.append(
                                    norm_units(b, o_ps,
                                               512 * jc_done - q0, jc_done,
                                               tail=is_last))

                    for kb in range(8 * half + 8):
                        k0 = base + 128 * kb
                        span_lo = max(q0, 128 * kb)      # batch-relative
                        span_w = q0 + 1024 - span_lo
                        pT = pwork.tile([128, 2, 1024], bf16, tag="pT",
                                        bufs=10, name=f"pT{b}{half}{kb}")
                        for seg in range(0, span_w, 512):
                            sw = min(512, span_w - seg)
                            for h in (0, 1):
                                r0 = 64 * h
                                sp = ps.tile([128, 512], f32, tag="sps",
                                             bufs=4,
                                             name=f"sp{b}{half}{kb}{seg}{h}")
                                nc.tensor.matmul(
                                    sp[:, 0:sw],
                                    qkvT[1][r0:r0 + 64, k0:k0 + 128],
                                    qkvT[0][r0:r0 + 64,
                                            base + span_lo + seg:
                                            base + span_lo + seg + sw],
                                    start=True, stop=True)
                                nc.scalar.activation(
                                    pT[:, h, seg:seg + sw],
                                    sp[:, 0:sw], Exp)
                            if seg == 0 and 128 * kb >= q0:
                                # the diag mask needs only seg0's exps;
                                # emitting it here lets DVE clear it before
                                # the next kb's O' reads pT
                                nc.vector.tensor_mul(
                                    pT[:, :, 0:128], pT[:, :, 0:128],
                                    maskm[:])
                        # O' for the previous kb runs while this kb's exp is
                        # still on ACT (breaks the per-kb PE->ACT->PE stall).
                        # pend carries across the half boundary so the last
                        # kb's O' overlaps the next half's first S batch.
                        if pend:
                            pe, pkb, ppT = pend.pop(0)
                            pe(pkb, ppT)
                        pend.append((emit_o, kb, pT))
                        drain_fillers(fill_rates[hi], proj_ok=proj_ok)
                for pe, pkb, ppT in pend:
                    pe(pkb, ppT)

            v_aug = [work.tile([128, NKB, 2, 65], bf16, tag=f"vaug{i}",
                               bufs=1, name=f"vaug{i}")
                     for i in range(B)]
            for b in range(B):
                nc.vector.tensor_copy(
                    v_aug[b][:, :, :, 64:65],
                    onecol_f[:, 0:1].to_broadcast((128, NKB, 2, 1)))
            # filler-drain rates per attention half, tuned on the
            # TimelineSim cost model: (attn0_h0, attn0_h1, attn1_h0,
            # attn1_h1, proj_ok_during_attn0)
            r = [1, 2, 2, 3, 0]
            fillerq0 = collections.deque()
            fillerq = collections.deque()
            projq = collections.deque()
            normq = collections.deque()
            qkv_batch(0, tchs=(0, 1))
            nc.sync.dma_start(
                wp_sb[:], wp_d.ap().rearrange("p (o n) -> p o n", n=128))
            # qkv(0) tch2/3 (needed from attention(0) half 1 on) and all of
            # qkv(1) run as fillers under attention(0) half 0
            fillerq0.append(qkv_units(0, tchs=(2, 3)))
            fillerq.append(qkv_units(1))
            attn_batch(0, fill_rates=(r[0], r[1]), proj_ok=bool(r[4]))
            # all remaining qkv(1) work must be emitted before
            # attention(1) consumes it
            drain_q(fillerq, 10 ** 6)
            attn_batch(1, fill_rates=(r[2], r[3]))
            drain_fillers(10 ** 6)
            drain_q(projq, 10 ** 6)

    nc.compile()
    return nc


def _prep_inputs(x, w_attn, b_attn, w_proj):
    import ml_dtypes
    bf16 = ml_dtypes.bfloat16
    xT = np.ascontiguousarray(x.reshape(BT, C).T.astype(bf16))
    scale = np.float32(1.0 / np.sqrt(HD))
    in_maps = []
    for c in range(NCORES):
        lo = 128 * c
        wq = w_attn[:, lo:lo + 128] * scale
        wk = w_attn[:, C + lo:C + lo + 128]
        wv = w_attn[:, 2 * C + lo:2 * C + lo + 128]
        wqkv = np.ascontiguousarray(
            np.concatenate([wq, wk, wv], axis=1).astype(bf16))
        bq = b_attn[lo:lo + 128] * scale
        bk = b_attn[C + lo:C + lo + 128]
        bv = b_attn[2 * C + lo:2 * C + lo + 128]
        bqkv = np.ascontiguousarray(
            np.stack([bq, bk, bv], axis=1).astype(np.float32))  # [128, 3]
        wp = np.ascontiguousarray(w_proj[lo:lo + 128, :].astype(bf16))
        in_maps.append({"xT": xT, "wqkv": wqkv, "bqkv": bqkv, "wp": wp})
    return in_maps


def kernel(x, w_attn, b_attn, w_proj, b_proj, _trace=False):
    from concourse.bass_utils import run_bass_kernel_spmd

    x = np.asarray(x, dtype=np.float32)
    w_attn = np.asarray(w_attn, dtype=np.float32)
    b_attn = np.asarray(b_attn, dtype=np.float32)
    w_proj = np.asarray(w_proj, dtype=np.float32)
    b_proj = np.asarray(b_proj, dtype=np.float32)

    if "nc" not in _CACHE:
        _CACHE["nc"] = _build_program()
    nc = _CACHE["nc"]

    in_maps = _prep_inputs(x, w_attn, b_attn, w_proj)
    res = run_bass_kernel_spmd(nc, in_maps, core_ids=list(range(NCORES)),
                               trace=_trace)
    _CACHE["last_results"] = res

    outT = res.results[0]["outT"].astype(np.float64)
    for c in range(1, NCORES):
        outT += res.results[c]["outT"]
    # V bias folded on host: y = y_attn + bv exactly (softmax weights sum
    # to 1), so out += bv @ w_proj lands in the bias term
    b_eff = b_proj + b_attn[2 * C:3 * C].astype(np.float64) @ \
        w_proj.astype(np.float64)
    out = outT.T.astype(np.float32) + b_eff[None, :].astype(np.float32)
    return out.reshape(B, T, C)


# revision 75
# speedup vs baseline: 1.0050x; 1.0050x over previous
"""Causal self-attention kernel for 8 Trainium2 NeuronCores.

Problem: B=2, T=2048, C=1024, H=16 heads (HD=64).
  qkv = x @ w_attn + b_attn ; causal softmax attention ; y @ w_proj + b_proj

Sharding: tensor-parallel over heads. Core c owns heads {2c, 2c+1} for both
batches. Each core computes Q/K/V for its heads (from full x), runs causal
attention, and produces a partial projection output
outT_c = (y_local @ w_proj[rows_c])^T in bf16. Host sums the 8 partials,
adds the bias, and transposes back.

Design notes (activations/weights bf16, PSUM accumulation f32; rel err
~4e-3 vs the 2e-2 gate):
  - x is passed host-transposed as xT [C, B*T] (bf16) so it streams as the
    moving operand of qkvT = w_sel^T @ xT. Only the Q bias is applied on
    device: the K bias is a softmax no-op (per-query constant), and the V
    bias is folded into b_proj on the host (softmax weights sum to 1).
  - V is computed directly in natural [token, head-dim] layout using x as
    the matmul stationary operand (no PE transposes), and lands in a
    per-batch v_aug [128, kb, head, 65] tile whose 65th column is ones so
    the O' matmul also produces the softmax denominators for free.
  - Attention uses the S^T layout: S^T[k,q] PSUM tiles [128, q-span<=512];
    exp runs on ACT straight into a per-kb bf16 pT [128, 2 heads, span]
    tile; the causal diag mask is one multiplicative DVE op per kb; no
    max-subtraction (scores are O(1), exp stays finite).
  - normalize: DVE reciprocal of the denominator row, GPSIMD
    partition_broadcast across the 64 y-rows (no PE in the chain, so
    queued normalize units cannot deadlock the in-order PE stream), DVE
    multiply. Head 1's multiply writes yT[64:128] via a partition-shifted
    output AP. The final chunk uses an idle-PE matmul broadcast instead.
  - Scheduling: attention(0) starts after only tch0/1 of qkv(0); the rest
    of qkv(0), all of qkv(1), and (held-back) projection units drain as
    fillers inside the attention loops under tuned rates, keeping PE fed
    where ACT (exp) is locally the bottleneck. outT stores pair two
    128-row blocks per DMA to halve SP dispatch serialization.
"""

import numpy as np

B, T, C, H = 2, 2048, 1024, 16
HD = C // H          # 64
NCORES = 8
HPC = H // NCORES    # 2 heads per core
BT = B * T           # 4096
NCB = C // 128       # 8 contraction blocks
NKB = T // 128       # 16 key blocks per batch
NJC = T // 512       # 4 query chunks of 512 per batch

_CACHE = {}


def _build_program():
    import collections

    import concourse.bacc as bacc
    import concourse.mybir as mybir
    import concourse.tile as tile
    from concourse.masks import make_upper_triangular

    f32 = mybir.dt.float32
    f32r = mybir.dt.float32r
    bf16 = mybir.dt.bfloat16
    Exp = mybir.ActivationFunctionType.Exp

    nc = bacc.Bacc("TRN2", target_bir_lowering=False, debug=False,
                   num_devices=NCORES)

    xT_d = nc.dram_tensor("xT", [C, BT], bf16, kind="ExternalInput")
    wqkv_d = nc.dram_tensor("wqkv", [C, 3 * 128], bf16, kind="ExternalInput")
    bqkv_d = nc.dram_tensor("bqkv", [128, 3], f32, kind="ExternalInput")
    wp_d = nc.dram_tensor("wp", [128, C], bf16, kind="ExternalInput")
    outT_d = nc.dram_tensor("outT", [C, BT], bf16, kind="ExternalOutput")

    with tile.TileContext(nc) as tc:
        with tc.tile_pool(name="const", bufs=1) as cst, \
             tc.tile_pool(name="big", bufs=1) as big, \
             tc.tile_pool(name="work", bufs=2) as work, \
             tc.tile_pool(name="pwork", bufs=3) as pwork, \
             tc.tile_pool(name="ps", bufs=1, space="PSUM") as ps:

            # ---- critical-path loads, in consumption order ----
            w_sb = cst.tile([128, NCB, 3 * 128], bf16, tag="w")
            _wr = wqkv_d.ap().rearrange("(cb p) n -> p cb n", p=128)
            nc.sync.dma_start(w_sb[:, 0:1, :], _wr[:, 0:1, :])

            xT_r = xT_d.ap().rearrange("(cb p) t -> p cb t", p=128)

            # first x chunk (tch0) sub0, then the rest of w (needed by the
            # 2nd matmul of the first accumulation), then tch0 sub1
            x0_sb = work.tile([128, NCB, 512], bf16, tag="x", bufs=3,
                              name="x0")
            # first x chunks ride the (startup-idle) ACT HWDGE queue so
            # their dispatch chain overlaps the weight loads on SP
            nc.scalar.dma_start(x0_sb[:, 0:1, 0:256], xT_r[:, 0:1, 0:256])
            nc.sync.dma_start(w_sb[:, 1:4, :], _wr[:, 1:4, :])
            nc.sync.dma_start(w_sb[:, 4:NCB, :], _wr[:, 4:NCB, :])
            nc.sync.dma_start(x0_sb[:, 1:4, 0:256], xT_r[:, 1:4, 0:256])
            nc.sync.dma_start(x0_sb[:, 4:NCB, 0:256], xT_r[:, 4:NCB, 0:256])
            bq_sb = cst.tile([128, 3], f32, tag="bq")
            nc.sync.dma_start(bq_sb[:], bqkv_d.ap())
            nc.sync.dma_start(x0_sb[:, :, 256:512], xT_r[:, :, 256:512])

            # ---- remaining constants (wp load deferred to post-qkv) ----
            wp_sb = cst.tile([128, NCB, 128], bf16, tag="wp")
            maskm_f = cst.tile([128, 128], f32, tag="maskmf")
            make_upper_triangular(nc, maskm_f[:], val=1.0, diag=True)
            # two adjacent copies so the h-merged [128, 2, 128] diag
            # multiply uses one contiguous operand
            maskm = cst.tile([128, 2, 128], bf16, tag="maskm")
            nc.vector.tensor_copy(maskm[:, 0, :], maskm_f[:])
            nc.vector.tensor_copy(maskm[:, 1, :], maskm_f[:])
            onecol_f = cst.tile([128, 1], f32, tag="onecol")
            nc.vector.memset(onecol_f[:], 1.0)
            ones64 = cst.tile([1, 64], f32, tag="ones64")
            nc.vector.memset(ones64[:], 1.0)
            # prewarm the ACT exp table set while ACT is otherwise idle,
            # so the ~2.7us table load is off the attention critical path
            warm = cst.tile([1, 2], f32, tag="warm")
            nc.scalar.activation(warm[:, 0:1], onecol_f[0:1, 0:1], Exp)

            # ---- persistent activations ----
            qkvT = [big.tile([128, BT], bf16, tag=f"qkvT{t}", name=f"qkvT{t}")
                    for t in range(2)]
            yT = big.tile([128, BT], bf16, tag="yT", name="yT")

            # K bias is dropped entirely (softmax is invariant to the
            # per-query constant q . bk), and the V bias is folded into
            # b_proj on the host (y = y_attn + bv exactly, softmax weights
            # sum to 1), so only the Q bias is applied on-device.
            def qkv_units(b, tchs=None):
                for tch in (tchs if tchs is not None
                            else range(4 * b, 4 * b + 4)):
                    tc0 = tch * 512
                    if tch == 0:
                        x_sb = x0_sb       # DMA already emitted above
                    else:
                        x_sb = work.tile([128, NCB, 512], bf16, tag="x",
                                         bufs=3, name=f"x{tch}")
                        for s in range(2):
                            nc.sync.dma_start(
                                x_sb[:, :, s * 256:(s + 1) * 256],
                                xT_r[:, :, tc0 + s * 256:tc0 + (s + 1) * 256])
                    split = 2 if tch == 0 else 1
                    sub = 512 // split
                    yield
                    for cht in range(2):
                        pq = ps.tile([128, 512], f32, tag="sps", bufs=4,
                                     name=f"pq{tch}{cht}")
                        for s in range(split):
                            for cb in range(NCB):
                                nc.tensor.matmul(
                                    pq[:, s * sub:(s + 1) * sub],
                                    w_sb[:, cb, cht * 128:(cht + 1) * 128],
                                    x_sb[:, cb, s * sub:(s + 1) * sub],
                                    start=(cb == 0), stop=(cb == NCB - 1))
                        if cht == 0:
                            nc.vector.tensor_scalar_add(
                                qkvT[0][:, tc0:tc0 + 512], pq[:],
                                bq_sb[:, 0:1])
                        else:
                            nc.vector.tensor_copy(
                                qkvT[1][:, tc0:tc0 + 512], pq[:])
                        yield
                    # V in natural [token, head-dim] layout: x as the
                    # stationary operand, wv as moving -> no PE transposes
                    pv = ps.tile([128, 4, 2, 64], f32, tag="sps", bufs=4,
                                 name=f"pv{tch}")
                    for blk in range(4):
                        for cb in range(NCB):
                            nc.tensor.matmul(
                                pv[:, blk, :, :],
                                x_sb[:, cb, blk * 128:(blk + 1) * 128],
                                w_sb[:, cb, 2 * 128:3 * 128],
                                start=(cb == 0), stop=(cb == NCB - 1))
                        yield
                    kb0 = (tch % 4) * 4
                    for blk in range(4):
                        nc.vector.tensor_copy(
                            v_aug[b][:, kb0 + blk, :, 0:64],
                            pv[:, blk, :, :])
                    yield

            def qkv_batch(b, tchs=None):
                for _ in qkv_units(b, tchs):
                    pass

            def proj_tile_units(b, tch, tail=False):
                # two 128-row output blocks share one osb tile and one DMA
                # (halves the SP dispatch serialization, tail especially)
                tc0 = tch * 512
                o_r = outT_d.ap().rearrange("(ob p) t -> p ob t", p=128)
                for ot in range(NCB):
                    if ot % 2 == 0:
                        osb = work.tile([128, 2, 512], bf16, tag="osb",
                                        bufs=6, name=f"osb{ot}{tch}")
                    pp = ps.tile([128, 512], f32, tag="sps", bufs=4,
                                 name=f"pp{ot}{tch}")
                    nc.tensor.matmul(pp[:], wp_sb[:, ot, :],
                                     yT[:, tc0:tc0 + 512],
                                     start=True, stop=True)
                    if tail and ot % 2 == 0:
                        # ACT is idle at the very end; splitting the copies
                        # across ACT/DVE halves the tail's serial chain
                        nc.scalar.copy(osb[:, ot % 2, :], pp[:])
                    else:
                        nc.vector.tensor_copy(osb[:, ot % 2, :], pp[:])
                    if ot % 2 == 1:
                        nc.sync.dma_start(
                            o_r[:, ot - 1:ot + 1, tc0:tc0 + 512],
                            osb[:])
                    yield

            def norm_units(b, o_ps, ocol, jc, tail=False):
                # normalize both heads, then hand the projection tiles to
                # projq (appending only after both normalizes are fully
                # emitted keeps the engine streams deadlock-free)
                for h in (0, 1):
                    normalize_jc(b, h, o_ps[h], ocol, jc, tail=tail)
                    yield
                projq.append(proj_tile_units(b, 4 * b + jc, tail=tail))

            def normalize_jc(b, h, o_ps, ocol, jc, c0=0, cw=512,
                             tail=False):
                # y^T cols [512jc, 512jc+512) (batch-rel) = O^T * (1/d).
                # Both heads accumulate at PSUM rows 0..64 (y + denom);
                # head 1's final multiply writes yT[64:128] via a
                # partition-shifted output AP (no partition-shift DMA).
                # The 1/d broadcast across the 64 y-rows runs on the idle
                # GPSIMD engine, so normalize uses no PE instructions at
                # all (keeps the queued norm+proj units deadlock-free).
                base = 2048 * b + 512 * jc + c0
                ylo = 0 if h == 0 else 64
                d_sb = work.tile([1, 512], f32, tag="dsb", bufs=4,
                                 name=f"d{b}{h}{jc}{c0}")
                with nc.allow_low_precision(
                        reason="softmax denominators (~1e-4)"):
                    nc.vector.reciprocal(d_sb[0:1, 0:cw],
                                         o_ps[64:65, ocol:ocol + cw])
                rec_sb = work.tile([64, 512], f32, tag="recsb", bufs=4,
                                   name=f"rec{b}{h}{jc}{c0}")
                if tail:
                    # PE and ACT are idle at the very end: broadcast 1/d via
                    # a PE matmul + ACT copy instead of the serial Pool path
                    recD = ps.tile([128, 512], f32, tag="sps", bufs=4,
                                   name=f"recD{b}{h}{jc}")
                    nc.tensor.matmul(recD[0:64, 0:cw], ones64[:],
                                     d_sb[0:1, 0:cw], start=True, stop=True)
                    nc.scalar.copy(rec_sb[0:64, 0:cw], recD[0:64, 0:cw])
                else:
                    nc.gpsimd.partition_broadcast(rec_sb[0:64, 0:cw],
                                                  d_sb[0:1, 0:cw])
                nc.vector.tensor_mul(
                    yT[ylo:ylo + 64, base:base + cw],
                    o_ps[0:64, ocol:ocol + cw], rec_sb[0:64, 0:cw])

            def drain_one(q):
                while q:
                    try:
                        next(q[0])
                        return True
                    except StopIteration:
                        q.popleft()
                return False

            def drain_q(q, n):
                for _ in range(n):
                    if not drain_one(q):
                        break

            def drain_fillers(n, proj_ok=True):
                # normalize units first (they release PSUM accumulators and
                # unblock downstream proj); then fillers; proj units only
                # when allowed -- holding proj back during attn(0) reserves
                # PE work for the filler-starved attn(1) windows
                for i in range(n):
                    if drain_one(normq):
                        continue
                    if drain_one(fillerq0):
                        continue
                    if drain_one(fillerq):
                        continue
                    if not (proj_ok and drain_one(projq)):
                        break

            def attn_batch(b, fill_rates=(1, 3), proj_ok=True,
                           halves=(0, 1), last_split=1):
                # Both heads processed together: head0 in PE rows 0-63,
                # head1 in rows 64-127 -> S matmul pairs run concurrently.
                # Query range split in two halves so both heads' O'
                # accumulators fit in PSUM (2 banks each).
                base = 2048 * b
                for hi, half in enumerate(halves):
                    if hi == 1:
                        # the second query half reads Q columns produced by
                        # the deferred qkv units -- emit them all first
                        drain_q(fillerq0, 10 ** 6)
                    q0 = 1024 * half
                    o_ps = [ps.tile([128, 1024], f32, tag="ops", bufs=2,
                                    name=f"o{b}{half}{h}") for h in (0, 1)]

                    def emit_o(kb, pT):
                        span_lo = max(q0, 128 * kb)
                        for h in (0, 1):
                            for jc in range(max(2 * half, kb // 4),
                                            2 * half + 2):
                                cs = max(512 * jc, 128 * kb)
                                width = 512 * (jc + 1) - cs
                                nc.tensor.matmul(
                                    o_ps[h][0:65, cs - q0:cs - q0 + width],
                                    v_aug[b][:, kb, h, :],
                                    pT[:, h, cs - span_lo:
                                       cs - span_lo + width],
                                    start=(kb == 0), stop=(kb == 4 * jc + 3))
                        if kb % 4 == 3:
                            jc_done = kb // 4
                            if jc_done >= 2 * half:
                                is_last = (b == 1 and hi == 1
                                           and kb == 8 * half + 7)
                                normq.append(
                                    norm_units(b, o_ps,
                                               512 * jc_done - q0, jc_done,
                                               tail=is_last))

                    pending = None
                    for kb in range(8 * half + 8):
                        k0 = base + 128 * kb
                        span_lo = max(q0, 128 * kb)      # batch-relative
                        span_w = q0 + 1024 - span_lo
                        pT = pwork.tile([128, 2, 1024], bf16, tag="pT",
                                        bufs=10, name=f"pT{b}{half}{kb}")
                        for seg in range(0, span_w, 512):
                            sw = min(512, span_w - seg)
                            for h in (0, 1):
                                r0 = 64 * h
                                sp = ps.tile([128, 512], f32, tag="sps",
                                             bufs=4,
                                             name=f"sp{b}{half}{kb}{seg}{h}")
                                nc.tensor.matmul(
                                    sp[:, 0:sw],
                                    qkvT[1][r0:r0 + 64, k0:k0 + 128],
                                    qkvT[0][r0:r0 + 64,
                                            base + span_lo + seg:
                                            base + span_lo + seg + sw],
                                    start=True, stop=True)
                                nc.scalar.activation(
                                    pT[:, h, seg:seg + sw],
                                    sp[:, 0:sw], Exp)
                            if seg == 0 and 128 * kb >= q0:
                                # the diag mask needs only seg0's exps;
                                # emitting it here lets DVE clear it before
                                # the next kb's O' reads pT
                                nc.vector.tensor_mul(
                                    pT[:, :, 0:128], pT[:, :, 0:128],
                                    maskm[:])
                        # O' for the previous kb runs while this kb's exp is
                        # still on ACT (breaks the per-kb PE->ACT->PE stall)
                        if pending is not None:
                            emit_o(*pending)
                        pending = (kb, pT)
                        drain_fillers(fill_rates[hi], proj_ok=proj_ok)
                    emit_o(*pending)

            v_aug = [work.tile([128, NKB, 2, 65], bf16, tag=f"vaug{i}",
                               bufs=1, name=f"vaug{i}")
                     for i in range(B)]
            for b in range(B):
                nc.vector.tensor_copy(
                    v_aug[b][:, :, :, 64:65],
                    onecol_f[:, 0:1].to_broadcast((128, NKB, 2, 1)))
            # filler-drain rates per attention half, tuned on the
            # TimelineSim cost model: (attn0_h0, attn0_h1, attn1_h0,
            # attn1_h1, proj_ok_during_attn0)
            r = [1, 2, 2, 3, 0]
            fillerq0 = collections.deque()
            fillerq = collections.deque()
            projq = collections.deque()
            normq = collections.deque()
            qkv_batch(0, tchs=(0, 1))
            nc.sync.dma_start(
                wp_sb[:], wp_d.ap().rearrange("p (o n) -> p o n", n=128))
            # qkv(0) tch2/3 (needed from attention(0) half 1 on) and all of
            # qkv(1) run as fillers under attention(0) half 0
            fillerq0.append(qkv_units(0, tchs=(2, 3)))
            fillerq.append(qkv_units(1))
            attn_batch(0, fill_rates=(r[0], r[1]), proj_ok=bool(r[4]))
            # all remaining qkv(1) work must be emitted before
            # attention(1) consumes it
            drain_q(fillerq, 10 ** 6)
            attn_batch(1, fill_rates=(r[2], r[3]))
            drain_fillers(10 ** 6)
            drain_q(projq, 10 ** 6)

    nc.compile()
    return nc


def _prep_inputs(x, w_attn, b_attn, w_proj):
    import ml_dtypes
    bf16 = ml_dtypes.bfloat16
    xT = np.ascontiguousarray(x.reshape(BT, C).T.astype(bf16))
    scale = np.float32(1.0 / np.sqrt(HD))
    in_maps = []
    for c in range(NCORES):
        lo = 128 * c
        wq = w_attn[:, lo:lo + 128] * scale
        wk = w_attn[:, C + lo:C + lo + 128]
        wv = w_attn[:, 2 * C + lo:2 * C + lo + 128]
        wqkv = np.ascontiguousarray(
            np.concatenate([wq, wk, wv], axis=1).astype(bf16))
        bq = b_attn[lo:lo + 128] * scale
        bk = b_attn[C + lo:C + lo + 128]
        bv = b_attn[2 * C + lo:2 * C + lo + 128]
        bqkv = np.ascontiguousarray(
            np.stack([bq, bk, bv], axis=1).astype(np.float32))  # [128, 3]
        wp = np.ascontiguousarray(w_proj[lo:lo + 128, :].astype(bf16))
        in_maps.append({"xT": xT, "wqkv": wqkv, "bqkv": bqkv, "wp": wp})
    return in_maps


def kernel(x, w_attn, b_attn, w_proj, b_proj, _trace=False):
    from concourse.bass_utils import run_bass_kernel_spmd

    x = np.asarray(x, dtype=np.float32)
    w_attn = np.asarray(w_attn, dtype=np.float32)
    b_attn = np.asarray(b_attn, dtype=np.float32)
    w_proj = np.asarray(w_proj, dtype=np.float32)
    b_proj = np.asarray(b_proj, dtype=np.float32)

    if "nc" not in _CACHE:
        _CACHE["nc"] = _build_program()
    nc = _CACHE["nc"]

    in_maps = _prep_inputs(x, w_attn, b_attn, w_proj)
    res = run_bass_kernel_spmd(nc, in_maps, core_ids=list(range(NCORES)),
                               trace=_trace)
    _CACHE["last_results"] = res

    outT = res.results[0]["outT"].astype(np.float64)
    for c in range(1, NCORES):
        outT += res.results[c]["outT"]
    # V bias folded on host: y = y_attn + bv exactly (softmax weights sum
    # to 1), so out += bv @ w_proj lands in the bias term
    b_eff = b_proj + b_attn[2 * C:3 * C].astype(np.float64) @ \
        w_proj.astype(np.float64)
    out = outT.T.astype(np.float32) + b_eff[None, :].astype(np.float32)
    return out.reshape(B, T, C)


# revision 85
# speedup vs baseline: 1.0105x; 1.0055x over previous
"""Causal self-attention kernel for 8 Trainium2 NeuronCores.

Problem: B=2, T=2048, C=1024, H=16 heads (HD=64).
  qkv = x @ w_attn + b_attn ; causal softmax attention ; y @ w_proj + b_proj

Sharding: tensor-parallel over heads. Core c owns heads {2c, 2c+1} for both
batches. Each core computes Q/K/V for its heads (from full x), runs causal
attention, and produces a partial projection output
outT_c = (y_local @ w_proj[rows_c])^T in bf16. Host sums the 8 partials,
adds the bias, and transposes back.

Design notes (activations/weights bf16, PSUM accumulation f32; rel err
~4e-3 vs the 2e-2 gate):
  - x is passed host-transposed as xT [C, B*T] (bf16) so it streams as the
    moving operand of qkvT = w_sel^T @ xT. Only the Q bias is applied on
    device: the K bias is a softmax no-op (per-query constant), and the V
    bias is folded into b_proj on the host (softmax weights sum to 1).
  - V is computed directly in natural [token, head-dim] layout using x as
    the matmul stationary operand (no PE transposes), and lands in a
    per-batch v_aug [128, kb, head, 65] tile whose 65th column is ones so
    the O' matmul also produces the softmax denominators for free.
  - Attention uses the S^T layout: S^T[k,q] PSUM tiles [128, q-span<=512];
    exp runs on ACT straight into a per-kb bf16 pT [128, 2 heads, span]
    tile; the causal diag mask is one multiplicative DVE op per kb; no
    max-subtraction (scores are O(1), exp stays finite).
  - normalize: DVE reciprocal of the denominator row, GPSIMD
    partition_broadcast across the 64 y-rows (no PE in the chain, so
    queued normalize units cannot deadlock the in-order PE stream), DVE
    multiply. Head 1's multiply writes yT[64:128] via a partition-shifted
    output AP. The final chunk uses an idle-PE matmul broadcast instead.
  - Scheduling: attention(0) starts after only tch0/1 of qkv(0); the rest
    of qkv(0), all of qkv(1), and (held-back) projection units drain as
    fillers inside the attention loops under tuned rates, keeping PE fed
    where ACT (exp) is locally the bottleneck. outT stores pair two
    128-row blocks per DMA to halve SP dispatch serialization.
"""

import numpy as np

B, T, C, H = 2, 2048, 1024, 16
HD = C // H          # 64
NCORES = 8
HPC = H // NCORES    # 2 heads per core
BT = B * T           # 4096
NCB = C // 128       # 8 contraction blocks
NKB = T // 128       # 16 key blocks per batch
NJC = T // 512       # 4 query chunks of 512 per batch

_CACHE = {}


def _build_program():
    import collections

    import concourse.bacc as bacc
    import concourse.mybir as mybir
    import concourse.tile as tile
    from concourse.masks import make_upper_triangular

    f32 = mybir.dt.float32
    f32r = mybir.dt.float32r
    bf16 = mybir.dt.bfloat16
    Exp = mybir.ActivationFunctionType.Exp

    nc = bacc.Bacc("TRN2", target_bir_lowering=False, debug=False,
                   num_devices=NCORES)

    xT_d = nc.dram_tensor("xT", [C, BT], bf16, kind="ExternalInput")
    wqkv_d = nc.dram_tensor("wqkv", [C, 3 * 128], bf16, kind="ExternalInput")
    bqkv_d = nc.dram_tensor("bqkv", [128, 3], f32, kind="ExternalInput")
    wp_d = nc.dram_tensor("wp", [128, C], bf16, kind="ExternalInput")
    outT_d = nc.dram_tensor("outT", [C, BT], bf16, kind="ExternalOutput")

    with tile.TileContext(nc) as tc:
        with tc.tile_pool(name="const", bufs=1) as cst, \
             tc.tile_pool(name="big", bufs=1) as big, \
             tc.tile_pool(name="work", bufs=2) as work, \
             tc.tile_pool(name="pwork", bufs=3) as pwork, \
             tc.tile_pool(name="ps", bufs=1, space="PSUM") as ps:

            # ---- critical-path loads, in consumption order ----
            w_sb = cst.tile([128, NCB, 3 * 128], bf16, tag="w")
            _wr = wqkv_d.ap().rearrange("(cb p) n -> p cb n", p=128)
            nc.sync.dma_start(w_sb[:, 0:1, :], _wr[:, 0:1, :])

            xT_r = xT_d.ap().rearrange("(cb p) t -> p cb t", p=128)

            # first x chunk (tch0) sub0, then the rest of w (needed by the
            # 2nd matmul of the first accumulation), then tch0 sub1
            x0_sb = work.tile([128, NCB, 512], bf16, tag="x", bufs=3,
                              name="x0")
            # first x chunks ride the (startup-idle) ACT HWDGE queue so
            # their dispatch chain overlaps the weight loads on SP
            nc.scalar.dma_start(x0_sb[:, 0:1, 0:256], xT_r[:, 0:1, 0:256])
            nc.sync.dma_start(w_sb[:, 1:4, :], _wr[:, 1:4, :])
            nc.sync.dma_start(w_sb[:, 4:NCB, :], _wr[:, 4:NCB, :])
            nc.sync.dma_start(x0_sb[:, 1:4, 0:256], xT_r[:, 1:4, 0:256])
            nc.sync.dma_start(x0_sb[:, 4:NCB, 0:256], xT_r[:, 4:NCB, 0:256])
            bq_sb = cst.tile([128, 3], f32, tag="bq")
            nc.sync.dma_start(bq_sb[:], bqkv_d.ap())
            nc.sync.dma_start(x0_sb[:, :, 256:512], xT_r[:, :, 256:512])

            # ---- remaining constants (wp load deferred to post-qkv) ----
            wp_sb = cst.tile([128, NCB, 128], bf16, tag="wp")
            maskm_f = cst.tile([128, 128], f32, tag="maskmf")
            make_upper_triangular(nc, maskm_f[:], val=1.0, diag=True)
            # two adjacent copies so the h-merged [128, 2, 128] diag
            # multiply uses one contiguous operand
            maskm = cst.tile([128, 2, 128], bf16, tag="maskm")
            nc.vector.tensor_copy(maskm[:, 0, :], maskm_f[:])
            nc.vector.tensor_copy(maskm[:, 1, :], maskm_f[:])
            onecol_f = cst.tile([128, 1], f32, tag="onecol")
            nc.vector.memset(onecol_f[:], 1.0)
            ones64f = cst.tile([1, 64], f32, tag="ones64f")
            nc.vector.memset(ones64f[:], 1.0)
            ones64 = cst.tile([1, 64], bf16, tag="ones64")
            nc.vector.tensor_copy(ones64[:], ones64f[:])
            # prewarm the ACT exp table set while ACT is otherwise idle,
            # so the ~2.7us table load is off the attention critical path
            warm = cst.tile([1, 2], f32, tag="warm")
            nc.scalar.activation(warm[:, 0:1], onecol_f[0:1, 0:1], Exp)

            # ---- persistent activations ----
            qkvT = [big.tile([128, BT], bf16, tag=f"qkvT{t}", name=f"qkvT{t}")
                    for t in range(2)]
            yT = big.tile([128, BT], bf16, tag="yT", name="yT")

            # K bias is dropped entirely (softmax is invariant to the
            # per-query constant q . bk), and the V bias is folded into
            # b_proj on the host (y = y_attn + bv exactly, softmax weights
            # sum to 1), so only the Q bias is applied on-device.
            def qkv_units(b, tchs=None):
                for tch in (tchs if tchs is not None
                            else range(4 * b, 4 * b + 4)):
                    tc0 = tch * 512
                    if tch == 0:
                        x_sb = x0_sb       # DMA already emitted above
                    else:
                        x_sb = work.tile([128, NCB, 512], bf16, tag="x",
                                         bufs=3, name=f"x{tch}")
                        for s in range(2):
                            nc.sync.dma_start(
                                x_sb[:, :, s * 256:(s + 1) * 256],
                                xT_r[:, :, tc0 + s * 256:tc0 + (s + 1) * 256])
                    split = 2 if tch == 0 else 1
                    sub = 512 // split
                    yield
                    for cht in range(2):
                        pq = ps.tile([128, 512], f32, tag="sps", bufs=4,
                                     name=f"pq{tch}{cht}")
                        for s in range(split):
                            for cb in range(NCB):
                                nc.tensor.matmul(
                                    pq[:, s * sub:(s + 1) * sub],
                                    w_sb[:, cb, cht * 128:(cht + 1) * 128],
                                    x_sb[:, cb, s * sub:(s + 1) * sub],
                                    start=(cb == 0), stop=(cb == NCB - 1))
                        if cht == 0:
                            nc.vector.tensor_scalar_add(
                                qkvT[0][:, tc0:tc0 + 512], pq[:],
                                bq_sb[:, 0:1])
                        else:
                            nc.vector.tensor_copy(
                                qkvT[1][:, tc0:tc0 + 512], pq[:])
                        yield
                    # V in natural [token, head-dim] layout: x as the
                    # stationary operand, wv as moving -> no PE transposes
                    pv = ps.tile([128, 4, 2, 64], f32, tag="sps", bufs=4,
                                 name=f"pv{tch}")
                    for blk in range(4):
                        for cb in range(NCB):
                            nc.tensor.matmul(
                                pv[:, blk, :, :],
                                x_sb[:, cb, blk * 128:(blk + 1) * 128],
                                w_sb[:, cb, 2 * 128:3 * 128],
                                start=(cb == 0), stop=(cb == NCB - 1))
                        yield
                    kb0 = (tch % 4) * 4
                    for blk in range(4):
                        nc.vector.tensor_copy(
                            v_aug[b][:, kb0 + blk, :, 0:64],
                            pv[:, blk, :, :])
                    yield

            def qkv_batch(b, tchs=None):
                for _ in qkv_units(b, tchs):
                    pass

            def proj_tile_units(b, tch, tail=False):
                # two 128-row output blocks share one osb tile and one DMA
                # (halves the SP dispatch serialization, tail especially)
                tc0 = tch * 512
                o_r = outT_d.ap().rearrange("(ob p) t -> p ob t", p=128)
                for ot in range(NCB):
                    if ot % 2 == 0:
                        osb = work.tile([128, 2, 512], bf16, tag="osb",
                                        bufs=6, name=f"osb{ot}{tch}")
                    pp = ps.tile([128, 512], f32, tag="sps", bufs=4,
                                 name=f"pp{ot}{tch}")
                    nc.tensor.matmul(pp[:], wp_sb[:, ot, :],
                                     yT[:, tc0:tc0 + 512],
                                     start=True, stop=True)
                    if tail and ot % 2 == 0:
                        # ACT is idle at the very end; splitting the copies
                        # across ACT/DVE halves the tail's serial chain
                        nc.scalar.copy(osb[:, ot % 2, :], pp[:])
                    else:
                        nc.vector.tensor_copy(osb[:, ot % 2, :], pp[:])
                    if ot % 2 == 1:
                        nc.sync.dma_start(
                            o_r[:, ot - 1:ot + 1, tc0:tc0 + 512],
                            osb[:])
                    yield

            def norm_units(b, o_ps, ocol, jc, tail=False):
                # normalize both heads, then hand the projection tiles to
                # projq (appending only after both normalizes are fully
                # emitted keeps the engine streams deadlock-free)
                for h in (0, 1):
                    normalize_jc(b, h, o_ps[h], ocol, jc, tail=tail)
                    yield
                projq.append(proj_tile_units(b, 4 * b + jc, tail=tail))

            def normalize_jc(b, h, o_ps, ocol, jc, c0=0, cw=512,
                             tail=False):
                # y^T cols [512jc, 512jc+512) (batch-rel) = O^T * (1/d).
                # Both heads accumulate at PSUM rows 0..64 (y + denom);
                # head 1's final multiply writes yT[64:128] via a
                # partition-shifted output AP (no partition-shift DMA).
                # The 1/d broadcast across the 64 y-rows runs on the idle
                # GPSIMD engine, so normalize uses no PE instructions at
                # all (keeps the queued norm+proj units deadlock-free).
                base = 2048 * b + 512 * jc + c0
                ylo = 0 if h == 0 else 64
                d_sb = work.tile([1, 512], bf16 if tail else f32, tag="dsb",
                                 bufs=4, name=f"d{b}{h}{jc}{c0}",
                                 padded_shape=[1, 1024])
                with nc.allow_low_precision(
                        reason="softmax denominators (~4e-3 in bf16 tail)"):
                    nc.vector.reciprocal(d_sb[0:1, 0:cw],
                                         o_ps[64:65, ocol:ocol + cw])
                rec_sb = work.tile([64, 512], f32, tag="recsb", bufs=4,
                                   name=f"rec{b}{h}{jc}{c0}")
                if tail:
                    # PE and ACT are idle at the very end: broadcast 1/d via
                    # a bf16 PE matmul + ACT copy instead of the serial Pool
                    # path (bf16 keeps the matmul at 1 cycle/col)
                    recD = ps.tile([128, 512], f32, tag="sps", bufs=4,
                                   name=f"recD{b}{h}{jc}")
                    nc.tensor.matmul(recD[0:64, 0:cw], ones64[:],
                                     d_sb[0:1, 0:cw], start=True, stop=True)
                    nc.scalar.copy(rec_sb[0:64, 0:cw], recD[0:64, 0:cw])
                else:
                    nc.gpsimd.partition_broadcast(rec_sb[0:64, 0:cw],
                                                  d_sb[0:1, 0:cw])
                nc.vector.tensor_mul(
                    yT[ylo:ylo + 64, base:base + cw],
                    o_ps[0:64, ocol:ocol + cw], rec_sb[0:64, 0:cw])

            def drain_one(q):
                while q:
                    try:
                        next(q[0])
                        return True
                    except StopIteration:
                        q.popleft()
                return False

            def drain_q(q, n):
                for _ in range(n):
                    if not drain_one(q):
                        break

            def drain_fillers(n, proj_ok=True):
                # normalize units first (they release PSUM accumulators and
                # unblock downstream proj); then fillers; proj units only
                # when allowed -- holding proj back during attn(0) reserves
                # PE work for the filler-starved attn(1) windows
                for i in range(n):
                    if drain_one(normq):
                        continue
                    if drain_one(fillerq0):
                        continue
                    if drain_one(fillerq):
                        continue
                    if not (proj_ok and drain_one(projq)):
                        break

            def attn_batch(b, fill_rates=(1, 3), proj_ok=True,
                           halves=(0, 1), last_split=1):
                # Both heads processed together: head0 in PE rows 0-63,
                # head1 in rows 64-127 -> S matmul pairs run concurrently.
                # Query range split in two halves so both heads' O'
                # accumulators fit in PSUM (2 banks each).
                base = 2048 * b
                for hi, half in enumerate(halves):
                    if hi == 1:
                        # the second query half reads Q columns produced by
                        # the deferred qkv units -- emit them all first
                        drain_q(fillerq0, 10 ** 6)
                    q0 = 1024 * half
                    o_ps = [ps.tile([128, 1024], f32, tag="ops", bufs=2,
                                    name=f"o{b}{half}{h}") for h in (0, 1)]

                    def emit_o(kb, pT):
                        span_lo = max(q0, 128 * kb)
                        for h in (0, 1):
                            for jc in range(max(2 * half, kb // 4),
                                            2 * half + 2):
                                cs = max(512 * jc, 128 * kb)
                                width = 512 * (jc + 1) - cs
                                nc.tensor.matmul(
                                    o_ps[h][0:65, cs - q0:cs - q0 + width],
                                    v_aug[b][:, kb, h, :],
                                    pT[:, h, cs - span_lo:
                                       cs - span_lo + width],
                                    start=(kb == 0), stop=(kb == 4 * jc + 3))
                        if kb % 4 == 3:
                            jc_done = kb // 4
                            if jc_done >= 2 * half:
                                is_last = (b == 1 and hi == 1
                                           and kb == 8 * half + 7)
                                normq.append(
                                    norm_units(b, o_ps,
                                               512 * jc_done - q0, jc_done,
                                               tail=is_last))

                    pending = None
                    for kb in range(8 * half + 8):
                        k0 = base + 128 * kb
                        span_lo = max(q0, 128 * kb)      # batch-relative
                        span_w = q0 + 1024 - span_lo
                        pT = pwork.tile([128, 2, 1024], bf16, tag="pT",
                                        bufs=10, name=f"pT{b}{half}{kb}")
                        for seg in range(0, span_w, 512):
                            sw = min(512, span_w - seg)
                            for h in (0, 1):
                                r0 = 64 * h
                                sp = ps.tile([128, 512], f32, tag="sps",
                                             bufs=4,
                                             name=f"sp{b}{half}{kb}{seg}{h}")
                                nc.tensor.matmul(
                                    sp[:, 0:sw],
                                    qkvT[1][r0:r0 + 64, k0:k0 + 128],
                                    qkvT[0][r0:r0 + 64,
                                            base + span_lo + seg:
                                            base + span_lo + seg + sw],
                                    start=True, stop=True)
                                nc.scalar.activation(
                                    pT[:, h, seg:seg + sw],
                                    sp[:, 0:sw], Exp)
                            if seg == 0 and 128 * kb >= q0:
                                # the diag mask needs only seg0's exps;
                                # emitting it here lets DVE clear it before
                                # the next kb's O' reads pT
                                nc.vector.tensor_mul(
                                    pT[:, :, 0:128], pT[:, :, 0:128],
                                    maskm[:])
                        # O' for the previous kb runs while this kb's exp is
                        # still on ACT (breaks the per-kb PE->ACT->PE stall)
                        if pending is not None:
                            emit_o(*pending)
                        pending = (kb, pT)
                        drain_fillers(fill_rates[hi], proj_ok=proj_ok)
                    emit_o(*pending)

            v_aug = [work.tile([128, NKB, 2, 65], bf16, tag=f"vaug{i}",
                               bufs=1, name=f"vaug{i}")
                     for i in range(B)]
            for b in range(B):
                nc.vector.tensor_copy(
                    v_aug[b][:, :, :, 64:65],
                    onecol_f[:, 0:1].to_broadcast((128, NKB, 2, 1)))
            # filler-drain rates per attention half, tuned on the
            # TimelineSim cost model: (attn0_h0, attn0_h1, attn1_h0,
            # attn1_h1, proj_ok_during_attn0)
            r = [1, 2, 2, 3, 0]
            fillerq0 = collections.deque()
            fillerq = collections.deque()
            projq = collections.deque()
            normq = collections.deque()
            qkv_batch(0, tchs=(0, 1))
            nc.sync.dma_start(
                wp_sb[:], wp_d.ap().rearrange("p (o n) -> p o n", n=128))
            # qkv(0) tch2/3 (needed from attention(0) half 1 on) and all of
            # qkv(1) run as fillers under attention(0) half 0
            fillerq0.append(qkv_units(0, tchs=(2, 3)))
            fillerq.append(qkv_units(1))
            attn_batch(0, fill_rates=(r[0], r[1]), proj_ok=bool(r[4]))
            # all remaining qkv(1) work must be emitted before
            # attention(1) consumes it
            drain_q(fillerq, 10 ** 6)
            attn_batch(1, fill_rates=(r[2], r[3]))
            drain_fillers(10 ** 6)
            drain_q(projq, 10 ** 6)

    nc.compile()
    return nc


def _prep_inputs(x, w_attn, b_attn, w_proj):
    import ml_dtypes
    bf16 = ml_dtypes.bfloat16
    xT = np.ascontiguousarray(x.reshape(BT, C).T.astype(bf16))
    scale = np.float32(1.0 / np.sqrt(HD))
    in_maps = []
    for c in range(NCORES):
        lo = 128 * c
        wq = w_attn[:, lo:lo + 128] * scale
        wk = w_attn[:, C + lo:C + lo + 128]
        wv = w_attn[:, 2 * C + lo:2 * C + lo + 128]
        wqkv = np.ascontiguousarray(
            np.concatenate([wq, wk, wv], axis=1).astype(bf16))
        bq = b_attn[lo:lo + 128] * scale
        bk = b_attn[C + lo:C + lo + 128]
        bv = b_attn[2 * C + lo:2 * C + lo + 128]
        bqkv = np.ascontiguousarray(
            np.stack([bq, bk, bv], axis=1).astype(np.float32))  # [128, 3]
        wp = np.ascontiguousarray(w_proj[lo:lo + 128, :].astype(bf16))
        in_maps.append({"xT": xT, "wqkv": wqkv, "bqkv": bqkv, "wp": wp})
    return in_maps


def kernel(x, w_attn, b_attn, w_proj, b_proj, _trace=False):
    from concourse.bass_utils import run_bass_kernel_spmd

    x = np.asarray(x, dtype=np.float32)
    w_attn = np.asarray(w_attn, dtype=np.float32)
    b_attn = np.asarray(b_attn, dtype=np.float32)
    w_proj = np.asarray(w_proj, dtype=np.float32)
    b_proj = np.asarray(b_proj, dtype=np.float32)

    if "nc" not in _CACHE:
        _CACHE["nc"] = _build_program()
    nc = _CACHE["nc"]

    in_maps = _prep_inputs(x, w_attn, b_attn, w_proj)
    res = run_bass_kernel_spmd(nc, in_maps, core_ids=list(range(NCORES)),
                               trace=_trace)
    _CACHE["last_results"] = res

    outT = res.results[0]["outT"].astype(np.float64)
    for c in range(1, NCORES):
        outT += res.results[c]["outT"]
    # V bias folded on host: y = y_attn + bv exactly (softmax weights sum
    # to 1), so out += bv @ w_proj lands in the bias term
    b_eff = b_proj + b_attn[2 * C:3 * C].astype(np.float64) @ \
        w_proj.astype(np.float64)
    out = outT.T.astype(np.float32) + b_eff[None, :].astype(np.float32)
    return out.reshape(B, T, C)


# revision 90
# speedup vs baseline: 1.0124x; 1.0020x over previous
"""Causal self-attention kernel for 8 Trainium2 NeuronCores.

Problem: B=2, T=2048, C=1024, H=16 heads (HD=64).
  qkv = x @ w_attn + b_attn ; causal softmax attention ; y @ w_proj + b_proj

Sharding: tensor-parallel over heads. Core c owns heads {2c, 2c+1} for both
batches. Each core computes Q/K/V for its heads (from full x), runs causal
attention, and produces a partial projection output
outT_c = (y_local @ w_proj[rows_c])^T in bf16. Host sums the 8 partials,
adds the bias, and transposes back.

Design notes (activations/weights bf16, PSUM accumulation f32; rel err
~4e-3 vs the 2e-2 gate):
  - x is passed host-transposed as xT [C, B*T] (bf16) so it streams as the
    moving operand of qkvT = w_sel^T @ xT. Only the Q bias is applied on
    device: the K bias is a softmax no-op (per-query constant), and the V
    bias is folded into b_proj on the host (softmax weights sum to 1).
  - V is computed directly in natural [token, head-dim] layout using x as
    the matmul stationary operand (no PE transposes), and lands in a
    per-batch v_aug [128, kb, head, 65] tile whose 65th column is ones so
    the O' matmul also produces the softmax denominators for free.
  - Attention uses the S^T layout: S^T[k,q] PSUM tiles [128, q-span<=512];
    exp runs on ACT straight into a per-kb bf16 pT [128, 2 heads, span]
    tile; the causal diag mask is one multiplicative DVE op per kb; no
    max-subtraction (scores are O(1), exp stays finite).
  - normalize: DVE reciprocal of the denominator row, GPSIMD
    partition_broadcast across the 64 y-rows (no PE in the chain, so
    queued normalize units cannot deadlock the in-order PE stream), DVE
    multiply. Head 1's multiply writes yT[64:128] via a partition-shifted
    output AP. The final chunk uses an idle-PE matmul broadcast instead.
  - Scheduling: attention(0) starts after only tch0/1 of qkv(0); the rest
    of qkv(0), all of qkv(1), and (held-back) projection units drain as
    fillers inside the attention loops under tuned rates, keeping PE fed
    where ACT (exp) is locally the bottleneck. outT stores pair two
    128-row blocks per DMA to halve SP dispatch serialization.
"""

import numpy as np

B, T, C, H = 2, 2048, 1024, 16
HD = C // H          # 64
NCORES = 8
HPC = H // NCORES    # 2 heads per core
BT = B * T           # 4096
NCB = C // 128       # 8 contraction blocks
NKB = T // 128       # 16 key blocks per batch
NJC = T // 512       # 4 query chunks of 512 per batch

_CACHE = {}


def _build_program():
    import collections

    import concourse.bacc as bacc
    import concourse.mybir as mybir
    import concourse.tile as tile
    from concourse.masks import make_upper_triangular

    f32 = mybir.dt.float32
    f32r = mybir.dt.float32r
    bf16 = mybir.dt.bfloat16
    Exp = mybir.ActivationFunctionType.Exp

    nc = bacc.Bacc("TRN2", target_bir_lowering=False, debug=False,
                   num_devices=NCORES)

    xT_d = nc.dram_tensor("xT", [C, BT], bf16, kind="ExternalInput")
    wqkv_d = nc.dram_tensor("wqkv", [C, 3 * 128], bf16, kind="ExternalInput")
    bqkv_d = nc.dram_tensor("bqkv", [128, 3], f32, kind="ExternalInput")
    wp_d = nc.dram_tensor("wp", [128, C], bf16, kind="ExternalInput")
    outT_d = nc.dram_tensor("outT", [C, BT], bf16, kind="ExternalOutput")

    with tile.TileContext(nc) as tc:
        with tc.tile_pool(name="const", bufs=1) as cst, \
             tc.tile_pool(name="big", bufs=1) as big, \
             tc.tile_pool(name="work", bufs=2) as work, \
             tc.tile_pool(name="pwork", bufs=3) as pwork, \
             tc.tile_pool(name="ps", bufs=1, space="PSUM") as ps:

            # ---- critical-path loads, in consumption order ----
            w_sb = cst.tile([128, NCB, 3 * 128], bf16, tag="w")
            _wr = wqkv_d.ap().rearrange("(cb p) n -> p cb n", p=128)
            nc.sync.dma_start(w_sb[:, 0:1, :], _wr[:, 0:1, :])

            xT_r = xT_d.ap().rearrange("(cb p) t -> p cb t", p=128)

            # first x chunk (tch0) sub0, then the rest of w (needed by the
            # 2nd matmul of the first accumulation), then tch0 sub1
            x0_sb = work.tile([128, NCB, 512], bf16, tag="x", bufs=3,
                              name="x0")
            # first x chunks ride the (startup-idle) ACT HWDGE queue so
            # their dispatch chain overlaps the weight loads on SP
            nc.scalar.dma_start(x0_sb[:, 0:1, 0:256], xT_r[:, 0:1, 0:256])
            nc.sync.dma_start(w_sb[:, 1:4, :], _wr[:, 1:4, :])
            nc.sync.dma_start(x0_sb[:, 1:4, 0:256], xT_r[:, 1:4, 0:256])
            nc.sync.dma_start(w_sb[:, 4:NCB, :], _wr[:, 4:NCB, :])
            nc.sync.dma_start(x0_sb[:, 4:NCB, 0:256], xT_r[:, 4:NCB, 0:256])
            bq_sb = cst.tile([128, 3], f32, tag="bq")
            nc.sync.dma_start(bq_sb[:], bqkv_d.ap())
            nc.sync.dma_start(x0_sb[:, :, 256:512], xT_r[:, :, 256:512])

            # ---- remaining constants (wp load deferred to post-qkv) ----
            wp_sb = cst.tile([128, NCB, 128], bf16, tag="wp")
            maskm_f = cst.tile([128, 128], f32, tag="maskmf")
            make_upper_triangular(nc, maskm_f[:], val=1.0, diag=True)
            # two adjacent copies so the h-merged [128, 2, 128] diag
            # multiply uses one contiguous operand
            maskm = cst.tile([128, 2, 128], bf16, tag="maskm")
            nc.vector.tensor_copy(maskm[:, 0, :], maskm_f[:])
            nc.vector.tensor_copy(maskm[:, 1, :], maskm_f[:])
            onecol_f = cst.tile([128, 1], f32, tag="onecol")
            nc.vector.memset(onecol_f[:], 1.0)
            ones64f = cst.tile([1, 64], f32, tag="ones64f")
            nc.vector.memset(ones64f[:], 1.0)
            ones64 = cst.tile([1, 64], bf16, tag="ones64")
            nc.vector.tensor_copy(ones64[:], ones64f[:])
            # prewarm the ACT exp table set while ACT is otherwise idle,
            # so the ~2.7us table load is off the attention critical path
            warm = cst.tile([1, 2], f32, tag="warm")
            nc.scalar.activation(warm[:, 0:1], onecol_f[0:1, 0:1], Exp)

            # ---- persistent activations ----
            qkvT = [big.tile([128, BT], bf16, tag=f"qkvT{t}", name=f"qkvT{t}")
                    for t in range(2)]
            yT = big.tile([128, BT], bf16, tag="yT", name="yT")

            # K bias is dropped entirely (softmax is invariant to the
            # per-query constant q . bk), and the V bias is folded into
            # b_proj on the host (y = y_attn + bv exactly, softmax weights
            # sum to 1), so only the Q bias is applied on-device.
            def qkv_units(b, tchs=None):
                for tch in (tchs if tchs is not None
                            else range(4 * b, 4 * b + 4)):
                    tc0 = tch * 512
                    if tch == 0:
                        x_sb = x0_sb       # DMA already emitted above
                    else:
                        x_sb = work.tile([128, NCB, 512], bf16, tag="x",
                                         bufs=3, name=f"x{tch}")
                        for s in range(2):
                            nc.sync.dma_start(
                                x_sb[:, :, s * 256:(s + 1) * 256],
                                xT_r[:, :, tc0 + s * 256:tc0 + (s + 1) * 256])
                    split = 2 if tch == 0 else 1
                    sub = 512 // split
                    yield
                    for cht in range(2):
                        pq = ps.tile([128, 512], f32, tag="sps", bufs=4,
                                     name=f"pq{tch}{cht}")
                        for s in range(split):
                            for cb in range(NCB):
                                nc.tensor.matmul(
                                    pq[:, s * sub:(s + 1) * sub],
                                    w_sb[:, cb, cht * 128:(cht + 1) * 128],
                                    x_sb[:, cb, s * sub:(s + 1) * sub],
                                    start=(cb == 0), stop=(cb == NCB - 1))
                        if cht == 0:
                            nc.vector.tensor_scalar_add(
                                qkvT[0][:, tc0:tc0 + 512], pq[:],
                                bq_sb[:, 0:1])
                        else:
                            nc.vector.tensor_copy(
                                qkvT[1][:, tc0:tc0 + 512], pq[:])
                        yield
                    # V in natural [token, head-dim] layout: x as the
                    # stationary operand, wv as moving -> no PE transposes
                    pv = ps.tile([128, 4, 2, 64], f32, tag="sps", bufs=4,
                                 name=f"pv{tch}")
                    for blk in range(4):
                        for cb in range(NCB):
                            nc.tensor.matmul(
                                pv[:, blk, :, :],
                                x_sb[:, cb, blk * 128:(blk + 1) * 128],
                                w_sb[:, cb, 2 * 128:3 * 128],
                                start=(cb == 0), stop=(cb == NCB - 1))
                        yield
                    kb0 = (tch % 4) * 4
                    for blk in range(4):
                        nc.vector.tensor_copy(
                            v_aug[b][:, kb0 + blk, :, 0:64],
                            pv[:, blk, :, :])
                    yield

            def qkv_batch(b, tchs=None):
                for _ in qkv_units(b, tchs):
                    pass

            def proj_tile_units(b, tch, tail=False):
                # two 128-row output blocks share one osb tile and one DMA
                # (halves the SP dispatch serialization, tail especially)
                tc0 = tch * 512
                o_r = outT_d.ap().rearrange("(ob p) t -> p ob t", p=128)
                for ot in range(NCB):
                    if ot % 2 == 0:
                        osb = work.tile([128, 2, 512], bf16, tag="osb",
                                        bufs=6, name=f"osb{ot}{tch}")
                    pp = ps.tile([128, 512], f32, tag="sps", bufs=4,
                                 name=f"pp{ot}{tch}")
                    nc.tensor.matmul(pp[:], wp_sb[:, ot, :],
                                     yT[:, tc0:tc0 + 512],
                                     start=True, stop=True)
                    if tail and ot % 2 == 0:
                        # ACT is idle at the very end; splitting the copies
                        # across ACT/DVE halves the tail's serial chain
                        nc.scalar.copy(osb[:, ot % 2, :], pp[:])
                    else:
                        nc.vector.tensor_copy(osb[:, ot % 2, :], pp[:])
                    if ot % 2 == 1:
                        nc.sync.dma_start(
                            o_r[:, ot - 1:ot + 1, tc0:tc0 + 512],
                            osb[:])
                    yield

            def norm_units(b, o_ps, ocol, jc, tail=False):
                # normalize both heads, then hand the projection tiles to
                # projq (appending only after both normalizes are fully
                # emitted keeps the engine streams deadlock-free)
                for h in (0, 1):
                    normalize_jc(b, h, o_ps[h], ocol, jc, tail=tail)
                    yield
                projq.append(proj_tile_units(b, 4 * b + jc, tail=tail))

            def normalize_jc(b, h, o_ps, ocol, jc, c0=0, cw=512,
                             tail=False):
                # y^T cols [512jc, 512jc+512) (batch-rel) = O^T * (1/d).
                # Both heads accumulate at PSUM rows 0..64 (y + denom);
                # head 1's final multiply writes yT[64:128] via a
                # partition-shifted output AP (no partition-shift DMA).
                # The 1/d broadcast across the 64 y-rows runs on the idle
                # GPSIMD engine, so normalize uses no PE instructions at
                # all (keeps the queued norm+proj units deadlock-free).
                base = 2048 * b + 512 * jc + c0
                ylo = 0 if h == 0 else 64
                d_sb = work.tile([1, 512], bf16 if tail else f32, tag="dsb",
                                 bufs=4, name=f"d{b}{h}{jc}{c0}",
                                 padded_shape=[1, 1024])
                with nc.allow_low_precision(
                        reason="softmax denominators (~4e-3 in bf16 tail)"):
                    nc.vector.reciprocal(d_sb[0:1, 0:cw],
                                         o_ps[64:65, ocol:ocol + cw])
                rec_sb = work.tile([64, 512], f32, tag="recsb", bufs=4,
                                   name=f"rec{b}{h}{jc}{c0}")
                if tail:
                    # PE and ACT are idle at the very end: broadcast 1/d via
                    # a bf16 PE matmul + ACT copy instead of the serial Pool
                    # path (bf16 keeps the matmul at 1 cycle/col)
                    recD = ps.tile([128, 512], f32, tag="sps", bufs=4,
                                   name=f"recD{b}{h}{jc}")
                    nc.tensor.matmul(recD[0:64, 0:cw], ones64[:],
                                     d_sb[0:1, 0:cw], start=True, stop=True)
                    nc.scalar.copy(rec_sb[0:64, 0:cw], recD[0:64, 0:cw])
                else:
                    nc.gpsimd.partition_broadcast(rec_sb[0:64, 0:cw],
                                                  d_sb[0:1, 0:cw])
                nc.vector.tensor_mul(
                    yT[ylo:ylo + 64, base:base + cw],
                    o_ps[0:64, ocol:ocol + cw], rec_sb[0:64, 0:cw])

            def drain_one(q):
                while q:
                    try:
                        next(q[0])
                        return True
                    except StopIteration:
                        q.popleft()
                return False

            def drain_q(q, n):
                for _ in range(n):
                    if not drain_one(q):
                        break

            def drain_fillers(n, proj_ok=True):
                # normalize units first (they release PSUM accumulators and
                # unblock downstream proj); then fillers; proj units only
                # when allowed -- holding proj back during attn(0) reserves
                # PE work for the filler-starved attn(1) windows
                for i in range(n):
                    if drain_one(normq):
                        continue
                    if drain_one(fillerq0):
                        continue
                    if drain_one(fillerq):
                        continue
                    if not (proj_ok and drain_one(projq)):
                        break

            def attn_batch(b, fill_rates=(1, 3), proj_ok=True,
                           halves=(0, 1), last_split=1):
                # Both heads processed together: head0 in PE rows 0-63,
                # head1 in rows 64-127 -> S matmul pairs run concurrently.
                # Query range split in two halves so both heads' O'
                # accumulators fit in PSUM (2 banks each).
                base = 2048 * b
                for hi, half in enumerate(halves):
                    if hi == 1:
                        # the second query half reads Q columns produced by
                        # the deferred qkv units -- emit them all first
                        drain_q(fillerq0, 10 ** 6)
                    q0 = 1024 * half
                    o_ps = [ps.tile([128, 1024], f32, tag="ops", bufs=2,
                                    name=f"o{b}{half}{h}") for h in (0, 1)]

                    def emit_o(kb, pT):
                        span_lo = max(q0, 128 * kb)
                        for h in (0, 1):
                            for jc in range(max(2 * half, kb // 4),
                                            2 * half + 2):
                                cs = max(512 * jc, 128 * kb)
                                width = 512 * (jc + 1) - cs
                                nc.tensor.matmul(
                                    o_ps[h][0:65, cs - q0:cs - q0 + width],
                                    v_aug[b][:, kb, h, :],
                                    pT[:, h, cs - span_lo:
                                       cs - span_lo + width],
                                    start=(kb == 0), stop=(kb == 4 * jc + 3))
                        if kb % 4 == 3:
                            jc_done = kb // 4
                            if jc_done >= 2 * half:
                                is_last = (b == 1 and hi == 1
                                           and kb == 8 * half + 7)
                                normq.append(
                                    norm_units(b, o_ps,
                                               512 * jc_done - q0, jc_done,
                                               tail=is_last))

                    pending = None
                    for kb in range(8 * half + 8):
                        k0 = base + 128 * kb
                        span_lo = max(q0, 128 * kb)      # batch-relative
                        span_w = q0 + 1024 - span_lo
                        pT = pwork.tile([128, 2, 1024], bf16, tag="pT",
                                        bufs=10, name=f"pT{b}{half}{kb}")
                        for seg in range(0, span_w, 512):
                            sw = min(512, span_w - seg)
                            for h in (0, 1):
                                r0 = 64 * h
                                sp = ps.tile([128, 512], f32, tag="sps",
                                             bufs=4,
                                             name=f"sp{b}{half}{kb}{seg}{h}")
                                nc.tensor.matmul(
                                    sp[:, 0:sw],
                                    qkvT[1][r0:r0 + 64, k0:k0 + 128],
                                    qkvT[0][r0:r0 + 64,
                                            base + span_lo + seg:
                                            base + span_lo + seg + sw],
                                    start=True, stop=True)
                                nc.scalar.activation(
                                    pT[:, h, seg:seg + sw],
                                    sp[:, 0:sw], Exp)
                            if seg == 0 and 128 * kb >= q0:
                                # the diag mask needs only seg0's exps;
                                # emitting it here lets DVE clear it before
                                # the next kb's O' reads pT
                                nc.vector.tensor_mul(
                                    pT[:, :, 0:128], pT[:, :, 0:128],
                                    maskm[:])
                        # O' for the previous kb runs while this kb's exp is
                        # still on ACT (breaks the per-kb PE->ACT->PE stall)
                        if pending is not None:
                            emit_o(*pending)
                        pending = (kb, pT)
                        # hold proj in the final kb of the run: DVE is
                        # in-order, and osb copies emitted here would delay
                        # the tail chunk's reciprocals by ~0.7us each
                        hold = (b == 1 and hi == 1 and kb >= 8 * half + 7)
                        drain_fillers(fill_rates[hi],
                                      proj_ok=proj_ok and not hold)
                    emit_o(*pending)

            v_aug = [work.tile([128, NKB, 2, 65], bf16, tag=f"vaug{i}",
                               bufs=1, name=f"vaug{i}")
                     for i in range(B)]
            for b in range(B):
                nc.vector.tensor_copy(
                    v_aug[b][:, :, :, 64:65],
                    onecol_f[:, 0:1].to_broadcast((128, NKB, 2, 1)))
            # filler-drain rates per attention half, tuned on the
            # TimelineSim cost model: (attn0_h0, attn0_h1, attn1_h0,
            # attn1_h1, proj_ok_during_attn0)
            r = [1, 2, 2, 3, 0]
            fillerq0 = collections.deque()
            fillerq = collections.deque()
            projq = collections.deque()
            normq = collections.deque()
            qkv_batch(0, tchs=(0, 1))
            nc.sync.dma_start(
                wp_sb[:], wp_d.ap().rearrange("p (o n) -> p o n", n=128))
            # qkv(0) tch2/3 (needed from attention(0) half 1 on) and all of
            # qkv(1) run as fillers under attention(0) half 0
            fillerq0.append(qkv_units(0, tchs=(2, 3)))
            fillerq.append(qkv_units(1))
            attn_batch(0, fill_rates=(r[0], r[1]), proj_ok=bool(r[4]))
            # all remaining qkv(1) work must be emitted before
            # attention(1) consumes it
            drain_q(fillerq, 10 ** 6)
            attn_batch(1, fill_rates=(r[2], r[3]))
            drain_fillers(10 ** 6)
            drain_q(projq, 10 ** 6)

    nc.compile()
    return nc


def _prep_inputs(x, w_attn, b_attn, w_proj):
    import ml_dtypes
    bf16 = ml_dtypes.bfloat16
    xT = np.ascontiguousarray(x.reshape(BT, C).T.astype(bf16))
    scale = np.float32(1.0 / np.sqrt(HD))
    in_maps = []
    for c in range(NCORES):
        lo = 128 * c
        wq = w_attn[:, lo:lo + 128] * scale
        wk = w_attn[:, C + lo:C + lo + 128]
        wv = w_attn[:, 2 * C + lo:2 * C + lo + 128]
        wqkv = np.ascontiguousarray(
            np.concatenate([wq, wk, wv], axis=1).astype(bf16))
        bq = b_attn[lo:lo + 128] * scale
        bk = b_attn[C + lo:C + lo + 128]
        bv = b_attn[2 * C + lo:2 * C + lo + 128]
        bqkv = np.ascontiguousarray(
            np.stack([bq, bk, bv], axis=1).astype(np.float32))  # [128, 3]
        wp = np.ascontiguousarray(w_proj[lo:lo + 128, :].astype(bf16))
        in_maps.append({"xT": xT, "wqkv": wqkv, "bqkv": bqkv, "wp": wp})
    return in_maps


def kernel(x, w_attn, b_attn, w_proj, b_proj, _trace=False):
    from concourse.bass_utils import run_bass_kernel_spmd

    x = np.asarray(x, dtype=np.float32)
    w_attn = np.asarray(w_attn, dtype=np.float32)
    b_attn = np.asarray(b_attn, dtype=np.float32)
    w_proj = np.asarray(w_proj, dtype=np.float32)
    b_proj = np.asarray(b_proj, dtype=np.float32)

    if "nc" not in _CACHE:
        _CACHE["nc"] = _build_program()
    nc = _CACHE["nc"]

    in_maps = _prep_inputs(x, w_attn, b_attn, w_proj)
    res = run_bass_kernel_spmd(nc, in_maps, core_ids=list(range(NCORES)),
                               trace=_trace)
    _CACHE["last_results"] = res

    outT = res.results[0]["outT"].astype(np.float64)
    for c in range(1, NCORES):
        outT += res.results[c]["outT"]
    # V bias folded on host: y = y_attn + bv exactly (softmax weights sum
    # to 1), so out += bv @ w_proj lands in the bias term
    b_eff = b_proj + b_attn[2 * C:3 * C].astype(np.float64) @ \
        w_proj.astype(np.float64)
    out = outT.T.astype(np.float32) + b_eff[None, :].astype(np.float32)
    return out.reshape(B, T, C)
